# revision 9
# baseline (speedup 1.0000x reference)
"""Bass/Tile kernel builder for distributed causal MHA with RoPE on 8 NeuronCores.

v2: per-batch pipelined A2A.  Head-pair per core (16 heads / 8 cores), both
batches on every core.  Core c assembles the full context for tokens
[512c, 512c+512) of EACH batch, so attention chunk j maps 1:1 to destination
core j and the AllToAll can run per batch: batch-0's A2A and output
projection overlap batch-1's projection/attention.  A2A payload is bf16.

All matmuls run bf16 at full PE rate.
"""

import sys

sys.path.insert(0, "/opt/trn_rl_repo")

import numpy as np
try:
    from ml_dtypes import bfloat16 as np_bf16
except ImportError:
    import jax.numpy as _jnp
    np_bf16 = _jnp.bfloat16
import concourse.bass as bass
import concourse.mybir as mybir
import concourse.tile as tile
from concourse import bacc
from concourse.masks import make_identity

F32 = mybir.dt.float32
F32R = mybir.dt.float32r
BF16 = mybir.dt.bfloat16

D_MODEL = 1024
NUM_HEADS = 16
DHEAD = 64
THETA = 10000.0
N_CORES = 8
B = 2


def build_nc(S, single_core=False, reps=1):
    """Build the SPMD Bass program (identical on all 8 cores)."""
    assert S == 4096
    CH = S // 8            # tokens per core per batch (512)
    NJ = S // 512          # number of 512-wide chunks per batch (8)
    NK = S // 128          # number of 128-tall sk tiles (32)

    import os
    rope_pe = False
    mask_dve = os.environ.get("KMASK", "dve") == "dve"
    nc = bacc.Bacc("TRN2", target_bir_lowering=False, debug=False,
                   num_devices=1 if single_core else N_CORES)

    # ---- I/O ----
    xt = nc.dram_tensor("xt", [B, D_MODEL, S], BF16, kind="ExternalInput")
    wq = nc.dram_tensor("wq", [D_MODEL, 128], BF16, kind="ExternalInput")
    wk = nc.dram_tensor("wk", [D_MODEL, 128], BF16, kind="ExternalInput")
    wv = nc.dram_tensor("wv", [D_MODEL, 128], BF16, kind="ExternalInput")
    wo = nc.dram_tensor("wo", [D_MODEL, D_MODEL], BF16, kind="ExternalInput")
    cosm = nc.dram_tensor("cosm", [128, S], BF16, kind="ExternalInput")
    sinm = nc.dram_tensor("sinm", [128, S], BF16, kind="ExternalInput")
    sel2 = nc.dram_tensor("sel2", [2, 128], BF16, kind="ExternalInput")
    trimask = nc.dram_tensor("trimask", [128, 8, 512], BF16,
                             kind="ExternalInput")
    sel14 = nc.dram_tensor("sel14", [14, 7, 128], BF16, kind="ExternalInput")
    # rows [0:512) = batch-0 tokens [512c, 512c+512), rows [512:1024) batch-1
    out = nc.dram_tensor("out", [2 * CH, D_MODEL], F32, kind="ExternalOutput")

    import contextlib
    with tile.TileContext(nc) as tc:
        rep_loop = (tc.For_i(0, reps, 1) if reps > 1
                    else contextlib.nullcontext())
        with (
            rep_loop,
            tc.tile_pool(name="persist", bufs=1) as pp,
            tc.tile_pool(name="dram", bufs=1, space="DRAM") as dram,
        ):
            # ---- persistent tiles + prefetches ----
            # weights first on the SP queue (first matmuls wait on them);
            # bulk tables go via the Pool queue
            wq_sb = pp.tile([128, 8, 128], BF16, name="wq_sb")
            wk_sb = pp.tile([128, 8, 128], BF16, name="wk_sb")
            wv_sb = pp.tile([128, 8, 128], BF16, name="wv_sb")
            wq_r = wq.rearrange("(g p) d -> p g d", p=128)
            nc.sync.dma_start(wq_sb[:, 0, :], wq_r[:, 0, :])
            onesc = pp.tile([128, 2], BF16, name="onesc")
            nc.vector.memset(onesc[:], 1.0)
            # Pool queue ordered by first-use time: rope tables' first chunk,
            # masks, remaining tables, then the late-needed Wo-phase tensors
            cos_sb = pp.tile([128, S], BF16, name="cos_sb")
            sin_sb = pp.tile([128, S], BF16, name="sin_sb")
            nc.gpsimd.dma_start(cos_sb[:, 0:512], cosm[:, 0:512])
            nc.gpsimd.dma_start(sin_sb[:, 0:512], sinm[:, 0:512])
            ident = pp.tile([128, 128], BF16, name="ident")
            make_identity(nc, ident[:])
            tm_sb = pp.tile([128, 8, 512], BF16, name="tm_sb")
            nc.gpsimd.dma_start(tm_sb[:], trimask[:])
            nc.gpsimd.dma_start(cos_sb[:, 512:], cosm[:, 512:])
            nc.gpsimd.dma_start(sin_sb[:, 512:], sinm[:, 512:])
            sel2_sb = pp.tile([2, 128], BF16, name="sel2_sb")
            nc.gpsimd.dma_start(sel2_sb[:], sel2[:])
            sel14_sb = pp.tile([14, 7, 128], BF16, name="sel14_sb")
            nc.gpsimd.dma_start(sel14_sb[:], sel14[:])
            wo_sb = pp.tile([128, 8, D_MODEL], BF16, name="wo_sb")
            nc.gpsimd.dma_start(wo_sb[:],
                                wo.rearrange("(g p) d -> p g d", p=128))

            # long-lived per-chunk q/k tiles and v tiles
            qp = tc.alloc_tile_pool(name="qkv", bufs=1)
            qt = [[qp.tile([128, 512], BF16, name=f"qt{b}_{j}")
                   for j in range(NJ)] for b in range(B)]
            kt = [[qp.tile([128, 512], BF16, name=f"kt{b}_{j}")
                   for j in range(NJ)] for b in range(B)]
            vsb = [[qp.tile([128, 130], BF16, name=f"v{b}_{st}")
                    for st in range(NK)] for b in range(B)]
            rp2 = tc.alloc_tile_pool(name="rope2", bufs=2)

            # per-batch A2A bounce buffers (bf16)
            # chunk layout: rows [0:65) = h0 ctx+denom, [65:130) = h1
            ib = [dram.tile([8, 130, CH], BF16, name=f"ib{b}") for b in range(B)]
            ob = [dram.tile([8, 130, CH], BF16, name=f"ob{b}") for b in range(B)]

            def proj_chunk(b, sc, xp, ps1, psv):
                """QKV projection + RoPE for one 512-token chunk."""
                s0 = 512 * sc
                xch = xp.tile([128, 8, 512], BF16, name="xch", tag="xch")
                xsrc = xt[b, :, s0:s0 + 512].rearrange("(g p) s -> p g s", p=128)
                if b == 0 and sc == 0:
                    # split so the very first matmul starts after block 0 lands;
                    # thread the remaining weight loads between x blocks
                    nc.sync.dma_start(xch[:, 0, :], xsrc[:, 0, :])
                    nc.sync.dma_start(wq_sb[:, 1:8, :], wq_r[:, 1:8, :])
                    for kk in range(1, 4):
                        nc.sync.dma_start(xch[:, kk, :], xsrc[:, kk, :])
                    nc.sync.dma_start(wk_sb[:],
                                      wk.rearrange("(g p) d -> p g d", p=128))
                    nc.sync.dma_start(wv_sb[:],
                                      wv.rearrange("(g p) d -> p g d", p=128))
                    for kk in range(4, 8):
                        nc.sync.dma_start(xch[:, kk, :], xsrc[:, kk, :])
                else:
                    nc.sync.dma_start(xch[:, 0:4, :], xsrc[:, 0:4, :])
                    nc.sync.dma_start(xch[:, 4:8, :], xsrc[:, 4:8, :])
                for ten, wsb in ((qt[b][sc], wq_sb), (kt[b][sc], wk_sb)):
                    ps = ps1.tile([128, 512], F32, name="pps", tag="rot")
                    for kk in range(8):
                        nc.tensor.matmul(ps[:], wsb[:, kk, :], xch[:, kk, :],
                                         start=(kk == 0), stop=(kk == 7))
                    nc.vector.tensor_copy(ten[:], ps[:])
                    t1_ = rp2.tile([128, 512], BF16, name="t1", tag="t1")
                    t2_ = rp2.tile([128, 512], BF16, name="t2", tag="t2")
                    t2s_ = rp2.tile([128, 512], BF16, name="t2s", tag="t2s")
                    nc.vector.tensor_mul(t1_[:], ten[:], cos_sb[:, s0:s0 + 512])
                    nc.vector.tensor_mul(t2_[:], ten[:], sin_sb[:, s0:s0 + 512])
                    for blk in range(4):
                        src2 = 32 * (blk ^ 1)
                        nc.gpsimd.dma_start(
                            t2s_[32 * blk:32 * blk + 32, :],
                            t2_[src2:src2 + 32, :])
                    nc.vector.tensor_add(ten[:], t1_[:], t2s_[:])
                vt_ps = ps1.tile([128, 512], F32, name="pps", tag="rot")
                for kk in range(8):
                    nc.tensor.matmul(vt_ps[:], wv_sb[:, kk, :], xch[:, kk, :],
                                     start=(kk == 0), stop=(kk == 7))
                vt_sb = xp.tile([128, 512], BF16, name="vt_sb", tag="vtsb")
                nc.vector.tensor_copy(vt_sb[:], vt_ps[:])
                for st in range(4):
                    v_ps = psv.tile([128, 128], BF16, name="v_ps", tag="v")
                    nc.tensor.transpose(
                        v_ps[:], vt_sb[:, 128 * st:128 * st + 128], ident[:])
                    vt = vsb[b][4 * sc + st]
                    vt3 = vt[:].rearrange("p (a b) -> p a b", a=2)
                    nc.vector.tensor_copy(
                        vt3[:, :, 64:65],
                        onesc[:].rearrange("p (a b) -> p a b", a=2))
                    nc.vector.tensor_copy(
                        vt3[:, :, 0:64],
                        v_ps[:].rearrange("p (a b) -> p a b", a=2))

            SPG = 2  # score-group width in 512-slots (PSUM: 2 banks/group)

            def attn_chunk(b, j, scp, avp, ptp, cxp):
                """Causal attention for one 512-query chunk; writes ib[b][j]."""
                nk = min(4 * j + 4, NK)
                nslot = 2 * nk
                ngroup = (nslot + SPG - 1) // SPG
                sc_t = [scp.tile([128, 512 * SPG], F32, name="sc_t", tag="sc")
                        for _ in range(ngroup)]
                pt_t = [ptp.tile([128, 512 * SPG], BF16, name="pt_t", tag="pt")
                        for _ in range(ngroup)]

                def slot_ap(tiles, s):
                    o = 512 * (s % SPG)
                    return tiles[s // SPG][:, o:o + 512]

                # band tile k (k >= 4j, offset d = k-4j) only has live
                # columns [128d, 512): trim scores/exp/mask/AV to that range.
                # The trimmed-away pt/sc regions are never written nor read.
                def trim_of(k):
                    return 128 * (k - 4 * j) if k >= 4 * j else 0

                for k in range(nk):
                    tr = trim_of(k)
                    for h in range(2):
                        s = 2 * k + h
                        hb = 64 * h
                        nc.tensor.matmul(
                            slot_ap(sc_t, s)[:, tr:512],
                            kt[b][k // 4][hb:hb + 64,
                                          128 * (k % 4):128 * (k % 4) + 128],
                            qt[b][j][hb:hb + 64, tr:512],
                            start=True, stop=True)
                s_lo = nslot - 8
                for g in range(ngroup):
                    w = min(512 * SPG, (nslot - SPG * g) * 512)
                    tr = trim_of((SPG * g) // 2)
                    if tr == 0:
                        nc.scalar.activation(
                            pt_t[g][:, 0:w], sc_t[g][:, 0:w],
                            mybir.ActivationFunctionType.Exp, scale=0.125)
                    else:
                        # band group: both slots share d; skip dead columns
                        # via a 2-level AP
                        pt3 = pt_t[g][:].rearrange("p (s q) -> p s q", s=SPG)
                        sc3 = sc_t[g][:].rearrange("p (s q) -> p s q", s=SPG)
                        nc.scalar.activation(
                            pt3[:, :, tr:512], sc3[:, :, tr:512],
                            mybir.ActivationFunctionType.Exp, scale=0.125)
                # causal mask: band = last 8 slots, grouped trimmed muls
                for g in range(s_lo // SPG, ngroup):
                    a = max(SPG * g, s_lo)
                    z = min(SPG * g + SPG, nslot)
                    tr = trim_of(a // 2)
                    pt3 = pt_t[g][:].rearrange("p (s q) -> p s q", s=SPG)
                    sl = pt3[:, (a % SPG):(a % SPG) + (z - a), tr:512]
                    nc.vector.tensor_mul(
                        sl, sl, tm_sb[:, a - s_lo:z - s_lo, tr:512])
                av = [avp.tile([65, 512], F32, name=f"av{h}", tag=f"av{h}")
                      for h in range(2)]
                for k in range(nk):
                    tr = trim_of(k)
                    for h in range(2):
                        nc.tensor.matmul(
                            av[h][:, tr:512], vsb[b][k][:, 65 * h:65 * h + 65],
                            slot_ap(pt_t, 2 * k + h)[:, tr:512],
                            start=(k == 0), stop=(k == nk - 1))
                for h in range(2):
                    cx = cxp.tile([65, 512], BF16, name="cx", tag="cx")
                    nc.vector.tensor_copy(cx[:], av[h][:])
                    nc.sync.dma_start(
                        ib[b][j, 65 * h:65 * h + 65, :], cx[:])
                if single_core or reps > 1:
                    # timed-mode A2A substitute, priced per chunk
                    nc.gpsimd.dma_start(ob[b][j], ib[b][j])

            def a2a(b):
                if single_core or reps > 1:
                    pass  # per-chunk copies emitted in attn_chunk
                else:
                    nc.gpsimd.collective_compute(
                        "AllToAll", mybir.AluOpType.bypass,
                        replica_groups=[list(range(8))],
                        ins=[ib[b].opt()], outs=[ob[b].opt()])

            def wo_pieces(b, wop, wops, bcps, osbp):
                """Output projection for batch b as a list of emitters."""
                state = {}
                pieces = []

                def recip_piece():
                    # chunks 1..7 first; chunk 0 (last to arrive) separately
                    rq17 = wop.tile([14, CH], BF16, name=f"rq17_{b}")
                    rsrc = ob[b][:].rearrange("c (a r) s -> (c a) r s", a=2)[
                        :, 64, :]
                    nc.sync.dma_start(rq17[:], rsrc[2:16, :])
                    with nc.allow_low_precision(
                            reason="bf16 1/denom, 0.4% rel err ok"):
                        nc.vector.reciprocal(rq17[:], rq17[:])
                    state["rq17"] = rq17
                    state["csts"] = [None] * 8
                pieces.append(recip_piece)

                def ctx_piece(t):
                    ctxf = wop.tile([128, CH], BF16, name=f"ctxf{b}_{t}")
                    nc.sync.dma_start(ctxf[0:64, :], ob[b][t, 0:64, :])
                    nc.sync.dma_start(ctxf[64:128, :], ob[b][t, 65:129, :])
                    bc = bcps.tile([128, CH], F32, name="bc", tag="bc")
                    if t == 0:
                        rq0 = wop.tile([2, CH], BF16, name=f"rq0_{b}")
                        rsrc = ob[b][:].rearrange(
                            "c (a r) s -> (c a) r s", a=2)[:, 64, :]
                        nc.sync.dma_start(rq0[:], rsrc[0:2, :])
                        with nc.allow_low_precision(
                                reason="bf16 1/denom, 0.4% rel err ok"):
                            nc.vector.reciprocal(rq0[:], rq0[:])
                        nc.tensor.matmul(bc[:], sel2_sb[:], rq0[:],
                                         start=True, stop=True)
                    else:
                        nc.tensor.matmul(bc[:], sel14_sb[:, t - 1, :],
                                         state["rq17"][:],
                                         start=True, stop=True)
                    cst = wop.tile([128, CH], BF16, name=f"cst{b}_{t}")
                    nc.vector.tensor_mul(cst[:], ctxf[:], bc[:])
                    state["csts"][t] = cst
                for t in list(range(1, 8)) + [0]:
                    pieces.append(lambda t=t: ctx_piece(t))

                def st_piece(st):
                    osb = osbp.tile([128, D_MODEL], F32, name="osb", tag="osb")
                    for m2 in range(2):
                        wo_ps = wops.tile([128, 512], F32, name="wo_ps", tag="wo")
                        ts = list(range(1, 8)) + [0]
                        for i_, t in enumerate(ts):
                            nc.tensor.matmul(
                                wo_ps[:],
                                state["csts"][t][:, 128 * st:128 * st + 128],
                                wo_sb[:, t, 512 * m2:512 * m2 + 512],
                                start=(i_ == 0), stop=(i_ == 7))
                        nc.vector.tensor_copy(
                            osb[:, 512 * m2:512 * m2 + 512], wo_ps[:])
                        nc.sync.dma_start(
                            out[512 * b + 128 * st:512 * b + 128 * st + 128,
                                512 * m2:512 * m2 + 512],
                            osb[:, 512 * m2:512 * m2 + 512])
                for st in range(4):
                    pieces.append(lambda st=st: st_piece(st))
                return pieces

            # ---- pipeline ----
            import os
            _ph = int(os.environ.get("KPHASES", "3"))
            # phase 1: proj(b0) alone
            with (
                tc.tile_pool(name="xchA", bufs=3) as xpA,
                tc.tile_pool(name="p1A", bufs=3, space="PSUM") as ps1A,
                tc.tile_pool(name="vpsA", bufs=2, space="PSUM") as psvA,
            ):
                for sc in range(NJ):
                    proj_chunk(0, sc, xpA, ps1A, psvA)
            if _ph >= 2:
                # phase 2: attn(b0) interleaved with proj(b1)
                with (
                    tc.tile_pool(name="scB", bufs=2, space="PSUM") as scpB,
                    tc.tile_pool(name="avB", bufs=1, space="PSUM") as avpB,
                    tc.tile_pool(name="ptB", bufs=3) as ptpB,
                    tc.tile_pool(name="cxB", bufs=4) as cxpB,
                    tc.tile_pool(name="xchB", bufs=2) as xpB,
                    tc.tile_pool(name="p1B", bufs=1, space="PSUM") as ps1B,
                    tc.tile_pool(name="vpsB", bufs=1, space="PSUM") as psvB,
                ):
                    for j in range(NJ):
                        attn_chunk(0, j, scpB, avpB, ptpB, cxpB)
                        proj_chunk(1, j, xpB, ps1B, psvB)
                if _ph >= 3:
                    a2a(0)
                # phase 3: attn(b1) DESC interleaved with Wo(b0)
                with (
                    tc.tile_pool(name="scC", bufs=2, space="PSUM") as scpC,
                    tc.tile_pool(name="avC", bufs=1, space="PSUM") as avpC,
                    tc.tile_pool(name="ptC", bufs=3) as ptpC,
                    tc.tile_pool(name="cxC", bufs=4) as cxpC,
                    tc.tile_pool(name="wopC", bufs=1) as wopC,
                    tc.tile_pool(name="wopsC", bufs=1, space="PSUM") as wopsC,
                    tc.tile_pool(name="bcC", bufs=1, space="PSUM") as bcpsC,
                    tc.tile_pool(name="osbC", bufs=2) as osbpC,
                ):
                    pieces = (wo_pieces(0, wopC, wopsC, bcpsC, osbpC)
                              if _ph >= 3 else [])
                    # spread the 13 Wo(b0) pieces over the first attn chunks
                    sched = {0: [0], 1: [1, 2, 3], 2: [4, 5, 6],
                             3: [7, 8], 4: [9, 10], 5: [11, 12]}
                    for idx, j in enumerate(range(NJ - 1, -1, -1)):
                        attn_chunk(1, j, scpC, avpC, ptpC, cxpC)
                        if _ph >= 3:
                            for pi in sched.get(idx, []):
                                pieces[pi]()
                if _ph >= 3:
                    a2a(1)
                    # phase 4: Wo(b1)
                    with (
                        tc.tile_pool(name="wopD", bufs=1) as wopD,
                        tc.tile_pool(name="wopsD", bufs=2, space="PSUM") as wopsD,
                        tc.tile_pool(name="bcD", bufs=1, space="PSUM") as bcpsD,
                        tc.tile_pool(name="osbD", bufs=2) as osbpD,
                    ):
                        for p in wo_pieces(1, wopD, wopsD, bcpsD, osbpD):
                            p()

            rp2.release()
            qp.release()

    nc.compile()
    return nc


# ---------------------------------------------------------------------------
# Host-side sharding / assembly
# ---------------------------------------------------------------------------

def _rope_tables(token_positions, S):
    half = DHEAD // 2
    inv_freq = THETA ** (-2.0 * np.arange(half, dtype=np.float32) / DHEAD)
    angles = np.arange(4096, dtype=np.float32)[:, None] * inv_freq[None, :]
    cos_c, sin_c = np.cos(angles), np.sin(angles)
    pos = np.asarray(token_positions).astype(np.int64)
    cosT = cos_c[pos].T.astype(np.float32)   # [32, S]
    sinT = sin_c[pos].T.astype(np.float32)
    cosm = np.concatenate([cosT, cosT, cosT, cosT], 0)
    sinm = np.concatenate([sinT, -sinT, sinT, -sinT], 0)
    return (np.ascontiguousarray(cosm).astype(np_bf16),
            np.ascontiguousarray(sinm).astype(np_bf16))


def prepare_in_maps(in_features, token_positions, Wq, Wk, Wv, Wo):
    Bb, S, D = in_features.shape
    xt = np.ascontiguousarray(in_features.transpose(0, 2, 1)).astype(np_bf16)
    cosm, sinm = _rope_tables(token_positions, S)
    sel2 = np.zeros((2, 128), np.float32)
    sel2[0, :64] = 1.0
    sel2[1, 64:] = 1.0
    sel2 = sel2.astype(np_bf16)
    pp_, ss_, qq_ = np.arange(128)[:, None, None], np.arange(8)[None, :, None], \
        np.arange(512)[None, None, :]
    trimask = (qq_ >= pp_ + 128 * (ss_ // 2)).astype(np_bf16)
    sel14 = np.zeros((14, 7, 128), np.float32)
    for t_ in range(1, 8):
        sel14[2 * (t_ - 1), t_ - 1, 0:64] = 1.0
        sel14[2 * (t_ - 1) + 1, t_ - 1, 64:128] = 1.0
    sel14 = sel14.astype(np_bf16)
    perm = np.concatenate([np.arange(0, 64, 2), np.arange(1, 64, 2)])
    woT = np.ascontiguousarray(Wo.T).astype(np_bf16)
    in_maps = []
    for c in range(N_CORES):
        h0, h1 = 2 * c, 2 * c + 1
        blocks_qk = []
        for W in (Wq, Wk):
            cols = []
            for h in (h0, h1):
                blk = W[64 * h:64 * h + 64, :][perm, :]   # [64, D] permuted
                cols.append(blk.T)                         # [D, 64]
            blocks_qk.append(np.ascontiguousarray(
                np.concatenate(cols, axis=1)).astype(np_bf16))
        wv_c = np.ascontiguousarray(np.concatenate(
            [Wv[64 * h:64 * h + 64, :].T for h in (h0, h1)],
            axis=1)).astype(np_bf16)
        in_maps.append({
            "xt": xt, "wq": blocks_qk[0], "wk": blocks_qk[1], "wv": wv_c,
            "wo": woT, "cosm": cosm, "sinm": sinm, "sel2": sel2,
            "trimask": trimask, "sel14": sel14,
        })
    return in_maps


def assemble(results, S):
    CH = S // 8
    out = np.zeros((B, S, D_MODEL), np.float32)
    for c in range(N_CORES):
        r = results[c]["out"]
        out[0, CH * c:CH * (c + 1), :] = r[0:CH]
        out[1, CH * c:CH * (c + 1), :] = r[CH:2 * CH]
    return out


from concourse.bass_utils import run_bass_kernel_spmd

_S = 4096
_NC = None


def _get_nc():
    global _NC
    if _NC is None:
        _NC = build_nc(_S)
    return _NC


def kernel(in_features, token_positions, Wq, Wk, Wv, Wo):
    x = np.asarray(in_features, dtype=np.float32)
    pos = np.asarray(token_positions)
    Wq = np.asarray(Wq, dtype=np.float32)
    Wk = np.asarray(Wk, dtype=np.float32)
    Wv = np.asarray(Wv, dtype=np.float32)
    Wo = np.asarray(Wo, dtype=np.float32)
    nc = _get_nc()
    in_maps = prepare_in_maps(x, pos, Wq, Wk, Wv, Wo)
    res = run_bass_kernel_spmd(nc, in_maps, list(range(N_CORES)))
    return assemble(res.results, _S)
